# revision 7
# baseline (speedup 1.0000x reference)
"""Cumulative (causal) normalization along time for x[16, 256, 8192] on 8 trn2 cores.

Strategy:
  - Shard the 4096 (B*C) rows across 8 cores (512 rows each).
  - Host pre-transposes each shard to [T=8192, rows=512], viewed as
    [64 chunks, 128 t, 512 rows], so time lies on SBUF partitions.
  - Cumsum(x) and cumsum(x^2) are computed per 128-t chunk with TensorEngine
    triangular matmuls (float32r = full PE rate); carries across chunks come
    from a running chunk-totals table via K-sliced ones-matrix matmuls.
    Elementwise normalization uses per-partition scalars (count columns) on
    ScalarE/VectorE/GPSIMD:
      out = (c*x - s) * exp(-0.5*ln(c*q - s^2 + eps*c^2)),  c = t+1.
  - The PE computes fp32/f32r products at ~16-bit-mantissa precision, whose
    noise is amplified by the c*q - s^2 cancellation only for small counts:
    t in [0, 128) is instead computed by an exact-fp32 fixup path (DVE
    tensor_tensor_scan in natural layout on a second small input copy),
    transposed back on the PE.
"""

import numpy as np

B, C, T = 16, 256, 8192
N_CORES = 8
ROWS_PER_CORE = (B * C) // N_CORES  # 512
P = 128                             # partitions / chunk height along T
CH = T // P                         # 64 chunks
RW = ROWS_PER_CORE                  # 512 rows = matmul free dim
G = 8                               # chunks per pipeline group
NG = CH // G                        # number of groups
RT = RW // P                        # fixup row-tiles (4)
EPS = 1e-4

_COMPILED = {}


def _build(reps: int, use_loop: bool):
    import concourse.bacc as bacc
    import concourse.mybir as mybir
    from concourse.tile import TileContext

    F32 = mybir.dt.float32
    F32R = mybir.dt.float32r
    A = mybir.AluOpType
    AF = mybir.ActivationFunctionType

    nc = bacc.Bacc("TRN2", target_bir_lowering=False, debug=False,
                   num_devices=N_CORES)

    x_d = nc.dram_tensor("x", [CH, P, RW], F32R, kind="ExternalInput").ap()
    x0n_d = nc.dram_tensor("x0nat", [RT, P, P], F32, kind="ExternalInput").ap()
    y_d = nc.dram_tensor("y", [CH, P, RW], F32, kind="ExternalOutput").ap()
    tri_d = nc.dram_tensor("tri", [P, P], F32R, kind="ExternalInput").ap()
    onesm_d = nc.dram_tensor("onesm", [CH, P], F32R, kind="ExternalInput").ap()
    stair_d = nc.dram_tensor("stair", [P, 2 * G], F32R, kind="ExternalInput").ap()
    ident_d = nc.dram_tensor("ident", [P, P], F32, kind="ExternalInput").ap()
    invc_d = nc.dram_tensor("invc", [P, P], F32, kind="ExternalInput").ap()
    ccol_d = nc.dram_tensor("ccol", [P, CH], F32, kind="ExternalInput").ap()
    epsc2_d = nc.dram_tensor("epsc2", [P, CH], F32, kind="ExternalInput").ap()

    with TileContext(nc) as tc:
        with (
            tc.tile_pool(name="consts", bufs=1) as cpool,
            tc.tile_pool(name="tots", bufs=1) as tpool,
            tc.tile_pool(name="stage", bufs=2) as stpool,
            tc.tile_pool(name="fix", bufs=2) as fpool,
            tc.tile_pool(name="fixout", bufs=1) as fopool,
            tc.tile_pool(name="xg", bufs=3) as xpool,
            tc.tile_pool(name="sqg", bufs=2) as sqpool,
            tc.tile_pool(name="s2g", bufs=2) as s2pool,
            tc.tile_pool(name="den2g", bufs=2) as dpool,
            tc.tile_pool(name="numg", bufs=2) as npool,
            tc.tile_pool(name="ps_s", bufs=3, space="PSUM") as pspool,
            tc.tile_pool(name="ps_q", bufs=2, space="PSUM") as pqpool,
            tc.tile_pool(name="ps_tot", bufs=1, space="PSUM") as ptpool,
        ):
            tri = cpool.tile([P, P], F32R)
            onesm = cpool.tile([CH, P], F32R)
            stair = cpool.tile([P, 2 * G], F32R)
            ident = cpool.tile([P, P], F32)
            invc = cpool.tile([P, P], F32)
            ccol = cpool.tile([P, CH], F32)
            epsc2 = cpool.tile([P, CH], F32)
            for t_, s_ in ((tri, tri_d), (onesm, onesm_d), (stair, stair_d),
                           (ident, ident_d), (invc, invc_d), (ccol, ccol_d),
                           (epsc2, epsc2_d)):
                nc.sync.dma_start(t_[:], s_[:])

            tots_s = tpool.tile([CH, RW], F32R, tag="tots_s")
            tots_q = tpool.tile([CH, RW], F32R, tag="tots_q")
            eps_col = cpool.tile([P, 1], F32)
            nc.vector.memset(eps_col[:], EPS)

            def fixup():
                """Exact-fp32 path for t in [0, 128): natural layout + DVE scans."""
                outT = fopool.tile([P, RW], F32, tag="fix_outT")
                for rt in range(RT):
                    xn = fpool.tile([P, P], F32, tag="fix_xn")
                    nc.sync.dma_start(xn[:], x0n_d[rt])
                    cs = fpool.tile([P, P], F32, tag="fix_cs")
                    nc.vector.tensor_tensor_scan(cs[:], xn[:], xn[:], 0.0,
                                                 A.add, A.bypass)
                    sqn = fpool.tile([P, P], F32, tag="fix_sqn")
                    nc.scalar.square(sqn[:], xn[:])
                    cq = fpool.tile([P, P], F32, tag="fix_cq")
                    nc.vector.tensor_tensor_scan(cq[:], sqn[:], sqn[:], 0.0,
                                                 A.add, A.bypass)
                    mean = fpool.tile([P, P], F32, tag="fix_mean")
                    nc.vector.tensor_tensor(mean[:], cs[:], invc[:], A.mult)
                    m2 = fpool.tile([P, P], F32, tag="fix_m2")
                    nc.vector.tensor_tensor(m2[:], cq[:], invc[:], A.mult)
                    msq = fpool.tile([P, P], F32, tag="fix_msq")
                    nc.scalar.square(msq[:], mean[:])
                    nc.vector.tensor_tensor(m2[:], m2[:], msq[:], A.subtract)
                    nc.scalar.activation(m2[:], m2[:], AF.Ln, bias=eps_col[:],
                                         scale=1.0)
                    nc.scalar.activation(m2[:], m2[:], AF.Exp, bias=0.0, scale=-0.5)
                    nc.vector.tensor_tensor(mean[:], xn[:], mean[:], A.subtract)
                    nc.vector.tensor_tensor(mean[:], mean[:], m2[:], A.mult)
                    pst = ptpool.tile([P, P], F32, tag="fix_ps")
                    nc.tensor.transpose(pst[:], mean[:], ident[:])
                    nc.scalar.copy(outT[:, rt * P:(rt + 1) * P], pst[:])
                nc.sync.dma_start(y_d[0], outT[:])

            def body(_=None):
                pt_s = ptpool.tile([G, RW], F32, tag="pt_s")
                pt_q = ptpool.tile([G, RW], F32, tag="pt_q")
                fixup()
                for g in range(NG):
                    # ---- stage A: load, square, chunk totals ----
                    xg = xpool.tile([P, G * RW], F32R)
                    nc.sync.dma_start(
                        xg[:].rearrange("p (c r) -> p c r", c=G),
                        x_d[g * G:(g + 1) * G].rearrange("c p r -> p c r"))
                    sqg = sqpool.tile([P, G * RW], F32R)
                    nc.gpsimd.tensor_tensor(sqg[:], xg[:], xg[:], A.mult)

                    # per-chunk column totals into psum rows 0..G-1 via a
                    # shifted ones-column lhsT (accumulating; other rows +0)
                    for j in range(G):
                        sl = slice(j * RW, (j + 1) * RW)
                        lhs = stair[:, G - j:2 * G - j]
                        nc.tensor.matmul(pt_s[:], lhs, xg[:, sl],
                                         start=(j == 0), stop=(j == G - 1))
                        nc.tensor.matmul(pt_q[:], lhs, sqg[:, sl],
                                         start=(j == 0), stop=(j == G - 1))
                    # evacuate to partition-0-based staging (same partitions),
                    # then DMA SBUF->SBUF into rows g*G.. of the totals table
                    stg_s = stpool.tile([G, RW], F32R, tag="stg_s")
                    stg_q = stpool.tile([G, RW], F32R, tag="stg_q")
                    nc.vector.tensor_copy(stg_s[:], pt_s[:])
                    nc.vector.tensor_copy(stg_q[:], pt_q[:])
                    nc.sync.dma_start(tots_s[g * G:(g + 1) * G, :], stg_s[:])
                    nc.sync.dma_start(tots_q[g * G:(g + 1) * G, :], stg_q[:])

                    # ---- stage B: cumsums + normalization ----
                    s2g = s2pool.tile([P, G * RW], F32)
                    den2g = dpool.tile([P, G * RW], F32)
                    numg = npool.tile([P, G * RW], F32)

                    for j in range(G):
                        c = g * G + j
                        if c == 0:
                            continue  # t<128 handled by the fixup path
                        sl = slice(j * RW, (j + 1) * RW)
                        xc = xg[:, sl]
                        sqc = sqg[:, sl]
                        ps_s = pspool.tile([P, RW], F32, tag="ps_s")
                        ps_q = pqpool.tile([P, RW], F32, tag="ps_q")
                        nc.tensor.matmul(ps_s[:], onesm[0:c, :], tots_s[0:c, :],
                                         start=True, stop=False)
                        nc.tensor.matmul(ps_s[:], tri[:], xc,
                                         start=False, stop=True)
                        nc.tensor.matmul(ps_q[:], onesm[0:c, :], tots_q[0:c, :],
                                         start=True, stop=False)
                        nc.tensor.matmul(ps_q[:], tri[:], sqc,
                                         start=False, stop=True)
                        # s2 = s^2
                        nc.scalar.square(s2g[:, sl], ps_s[:])
                        # den2 = c*q - s^2
                        nc.vector.scalar_tensor_tensor(
                            den2g[:, sl], ps_q[:], ccol[:, c:c + 1], s2g[:, sl],
                            A.mult, A.subtract)
                        # ln(den2 + eps*c^2)   (per-partition bias)
                        nc.scalar.activation(den2g[:, sl], den2g[:, sl], AF.Ln,
                                             bias=epsc2[:, c:c + 1], scale=1.0)
                        # num = c*x - s
                        nc.vector.scalar_tensor_tensor(
                            numg[:, sl], xc.bitcast(F32), ccol[:, c:c + 1], ps_s[:],
                            A.mult, A.subtract)
                    # rstd = exp(-0.5 * ln(...)), batched over the group
                    osl = slice(RW, G * RW) if g == 0 else slice(0, G * RW)
                    nc.scalar.activation(den2g[:, osl], den2g[:, osl], AF.Exp,
                                         bias=0.0, scale=-0.5)
                    # out = num * rstd (in place into numg); split DVE/GPSIMD
                    if g % 2 == 0:
                        nc.gpsimd.tensor_tensor(numg[:, osl], numg[:, osl],
                                                den2g[:, osl], A.mult)
                    else:
                        nc.vector.tensor_tensor(numg[:, osl], numg[:, osl],
                                                den2g[:, osl], A.mult)
                    c0 = g * G + (1 if g == 0 else 0)
                    nc.sync.dma_start(
                        y_d[c0:(g + 1) * G].rearrange("c p r -> p c r"),
                        numg[:, c0 * RW - g * G * RW:G * RW].rearrange(
                            "p (c r) -> p c r", c=(g + 1) * G - c0))

            if use_loop:
                with tc.For_i(0, reps, 1, hint_engines=(mybir.EngineType.PE,)):
                    body()
            else:
                body()

    nc.compile()
    return nc


def _host_consts():
    tri = np.triu(np.ones((P, P), dtype=np.float32))          # tri[t, t'] = t<=t'
    onesm = np.ones((CH, P), dtype=np.float32)
    stair = np.zeros((P, 2 * G), dtype=np.float32)            # ones col at G
    stair[:, G] = 1.0
    ident = np.eye(P, dtype=np.float32)
    invc = np.broadcast_to(
        1.0 / np.arange(1, P + 1, dtype=np.float64), (P, P)).astype(np.float32)
    t_global = (np.arange(P).reshape(P, 1) +
                P * np.arange(CH).reshape(1, CH)).astype(np.float64)
    ccol = (t_global + 1.0).astype(np.float32)                # [P, CH] counts
    epsc2 = (EPS * (t_global + 1.0) ** 2).astype(np.float32)  # [P, CH]
    return {"tri": tri, "onesm": onesm, "stair": stair, "ident": ident,
            "invc": invc, "ccol": ccol, "epsc2": epsc2}


def _get_compiled(reps: int, use_loop: bool = False):
    key = (reps, use_loop)
    if key not in _COMPILED:
        _COMPILED[key] = _build(reps, use_loop)
    return _COMPILED[key]


def _make_in_maps(x: np.ndarray):
    consts = _host_consts()
    xs = x.reshape(N_CORES, ROWS_PER_CORE, T)
    xr = np.ascontiguousarray(xs.transpose(0, 2, 1)).reshape(N_CORES, CH, P, RW)
    x0n = np.ascontiguousarray(
        xs[:, :, :P]).reshape(N_CORES, RT, P, P)
    return [{"x": xr[i], "x0nat": x0n[i], **consts} for i in range(N_CORES)]


def _gather(results) -> np.ndarray:
    ys = np.stack([results[i]["y"] for i in range(N_CORES)])  # [8, CH, P, RW]
    y = ys.reshape(N_CORES, T, RW).transpose(0, 2, 1)         # [8, RW, T]
    return np.ascontiguousarray(y).reshape(B, C, T)


def kernel(x: np.ndarray) -> np.ndarray:
    from concourse.bass_utils import run_bass_kernel_spmd

    x = np.asarray(x, dtype=np.float32)
    nc = _get_compiled(1, use_loop=False)
    res = run_bass_kernel_spmd(nc, _make_in_maps(x), list(range(N_CORES)))
    return _gather(res.results)
